# revision 1
# baseline (speedup 1.0000x reference)
"""Trainium2 Bass kernel for AdaptiveLogSoftmaxWithLoss (moe_routing).

Sharding: the three class dimensions are zero-padded and tensor-sharded
across the 8 cores (head 4002->4096, tail0 16000->16384, tail1
30257->30720), so every core runs an identical SPMD program over all 2048
samples with 1/8 of the output classes (6400 columns).

Per core:
  - hidden projections h0T=[512,2048], h1T=[256,2048] in transposed layout
    (fp8 DoubleRow GEMMs, inp scaled 16x / w1 64x), cast to bf16 (for the
    target dots) and to fp8*8 (as lhsT of the tail GEMMs),
  - logit shards computed in [sample, class] PSUM groups up to 4 banks wide
    (fp8 DoubleRow; head also fp8), one ACT exp (+accum_out, descaled via
    the activation scale) per group -> partial per-row sum-exp.  Logits are
    small by construction (|x| < ~4) so no max subtraction is needed,
  - target logits: the head uses a fused DVE (iota==rel)*logit pass on its
    PSUM group; the tails dot bf16 natural-layout hidden rows (batched XBAR
    DMA transposes of hT) against host-gathered target weight rows that are
    zeroed on non-owner cores,
  - emission order interleaves head groups with hidden0 blocks and hidden1
    blocks into the tail0 loop so the scalar engine (the exp bottleneck,
    ~13M elements/core) stays fed while the PE runs GEMMs.

Host combine: sum partials over cores, subtract the exact exp(0)=1
contribution of the zero-padded columns, lse = log(sum), gathers sum to the
single owner value, then NLL = -(head + masked tail terms) as in the
reference.  All heavy math (GEMMs, exp, reductions, gathers) runs on
device; the host only shards, pads, quantizes, and combines [N]-vectors.
"""

import numpy as np
import ml_dtypes

import concourse.bass as bass
import concourse.bacc as bacc
import concourse.mybir as mybir
import concourse.tile as tile
from concourse.bass_utils import run_bass_kernel_spmd

BF16 = ml_dtypes.bfloat16
FP8 = ml_dtypes.float8_e4m3
H_SCALE = 8.0     # h cast to fp8 at 8x
W_SCALE = 64.0    # tail w2 cast to fp8 at 64x
IN_SCALE = 16.0   # inp cast to fp8 at 16x
W1_SCALE = 64.0   # w1 / head_w cast to fp8 at 64x
HID_DESCALE = 1.0 / (IN_SCALE * W1_SCALE)
NCORES = 8
N, D = 2048, 1024
H0, H1 = 512, 256
C0, C1 = 4000, 20000
HEAD = 4002        # 4000 shortlist + 2 cluster-logit columns
HEAD_PAD = 4096    # padded so 8 cores get 512 each
T0 = 16000
T0_PAD = 16000     # divides by 8 exactly (2000 each, no padding)
T1 = 30257
T1_PAD = 30720     # padded so 8 cores get 3840 each
WH, W0, W1 = HEAD_PAD // 8, T0_PAD // 8, T1_PAD // 8   # 512, 2000, 3840
MT = N // 128                                          # 16 sample tiles
PAD_H = HEAD_PAD - HEAD   # 94 zero columns, all on core 7
PAD_0 = T0_PAD - T0       # 384 zero columns, all on core 7
PAD_1 = T1_PAD - T1       # 463 zero columns, all on core 7

# module-level knobs for test.py (harness never touches these)
TRACE = False
LAST_RESULT = None

_CACHED_NC = None


def _build_nc():
    nc = bacc.Bacc(None)
    BF = mybir.dt.bfloat16
    F8 = mybir.dt.float8e4
    F32 = mybir.dt.float32
    AX = mybir.AxisListType
    OP = mybir.AluOpType
    ACTF = mybir.ActivationFunctionType

    inpT_d = nc.dram_tensor("inpT", [128, D // 128, N], F8, kind="ExternalInput")
    w1t0_d = nc.dram_tensor("w1t0", [128, D // 128, H0], F8, kind="ExternalInput")
    w1t1_d = nc.dram_tensor("w1t1", [128, D // 128, H1], F8, kind="ExternalInput")
    hwT_d = nc.dram_tensor("hwT", [128, D // 128, WH], F8, kind="ExternalInput")
    w2t0_d = nc.dram_tensor("w2t0", [128, H0 // 128, W0], F8, kind="ExternalInput")
    w2t1_d = nc.dram_tensor("w2t1", [128, H1 // 128, W1], F8, kind="ExternalInput")
    wg0_d = nc.dram_tensor("wg0", [128, MT, H0], BF, kind="ExternalInput")
    wg1_d = nc.dram_tensor("wg1", [128, MT, H1], BF, kind="ExternalInput")
    iota_d = nc.dram_tensor("iota", [128, WH], F32, kind="ExternalInput")
    rels_d = nc.dram_tensor("rels", [128, MT, 3], F32, kind="ExternalInput")
    res_d = nc.dram_tensor("res", [128, MT, 6], F32, kind="ExternalOutput")

    with tile.TileContext(nc) as tc:
        with (
            tc.tile_pool(name="const", bufs=1) as cp,
            tc.tile_pool(name="work", bufs=3) as wp,
            tc.tile_pool(name="parts", bufs=4) as pp,
        ):
            inpT = cp.tile([128, D // 128, N], F8)
            w1t0 = cp.tile([128, D // 128, H0], F8)
            w1t1 = cp.tile([128, D // 128, H1], F8)
            hwT = cp.tile([128, D // 128, WH], F8)
            w2t0 = cp.tile([128, H0 // 128, W0], F8)
            w2t1 = cp.tile([128, H1 // 128, W1], F8)
            wg0 = cp.tile([128, MT, H0], BF)
            wg1 = cp.tile([128, MT, H1], BF)
            iota = cp.tile([128, WH], F32)
            rels = cp.tile([128, MT, 3], F32)
            h0T = cp.tile([128, H0 // 128, N], BF)
            h1T = cp.tile([128, H1 // 128, N], BF)
            h0T8 = cp.tile([128, H0 // 128, N], F8)
            h1T8 = cp.tile([128, H1 // 128, N], F8)
            h0n = cp.tile([128, MT, H0], BF)
            h1n = cp.tile([128, MT, H1], BF)
            res = cp.tile([128, MT, 6], F32)

            # loads ordered to match emission: head first, then hidden
            for kt in range(D // 128):
                nc.sync.dma_start(inpT[:, kt], inpT_d[:, kt])
                nc.sync.dma_start(hwT[:, kt], hwT_d[:, kt])
            nc.sync.dma_start(iota[:], iota_d[:])
            nc.sync.dma_start(rels[:], rels_d[:])
            nc.sync.dma_start(w1t0[:], w1t0_d[:])
            nc.sync.dma_start(w1t1[:], w1t1_d[:])
            nc.sync.dma_start(w2t0[:], w2t0_d[:])
            nc.sync.dma_start(wg0[:], wg0_d[:])
            nc.sync.dma_start(w2t1[:], w2t1_d[:])
            nc.sync.dma_start(wg1[:], wg1_d[:])

            # Front phase (head + hidden0) uses 6 one-bank slots; the
            # mid/tail phases use 2 four-bank slots.  The pools are opened
            # sequentially (the phase boundary is already data-serialized
            # on h0T8, so the pool swap costs nothing).
            fpool_cm = tc.tile_pool(name="psumF", bufs=6, space="PSUM")
            fpool = fpool_cm.__enter__()
            psp = None

            def fslot(w):
                ps = fpool.tile([128, 512], F32, tag="front", name="ps")
                return ps[:, :w]

            def pslot(w):
                ps = psp.tile([128, 2048], F32, tag="logits", name="ps")
                return ps[:, :w]

            DESCALE = 1.0 / (H_SCALE * W_SCALE)
            DR = mybir.MatmulPerfMode.DoubleRow

            def hidden_block(hT, hT8, w1, hdim, mh, alloc):
                # one h k-tile: [128 h, 2048 samples] in 512-col chunks
                for rc in range(N // 512):
                    ps = alloc(512)
                    for kt in range(0, D // 128, 2):
                        nc.tensor.matmul(
                            ps[:],
                            w1[:, kt : kt + 2, mh * 128 : (mh + 1) * 128],
                            inpT[:, kt : kt + 2, rc * 512 : (rc + 1) * 512],
                            start=(kt == 0),
                            stop=(kt + 2 >= D // 128),
                            perf_mode=DR,
                        )
                    nc.vector.tensor_scalar_mul(
                        hT[:, mh, rc * 512 : (rc + 1) * 512], ps[:], HID_DESCALE
                    )
                    nc.vector.tensor_scalar_mul(
                        hT8[:, mh, rc * 512 : (rc + 1) * 512],
                        hT[:, mh, rc * 512 : (rc + 1) * 512],
                        H_SCALE,
                    )

            def head_group(m):
                ms = slice(m * 128, (m + 1) * 128)
                ps = fslot(WH)
                for kt in range(0, D // 128, 2):
                    nc.tensor.matmul(
                        ps[:],
                        inpT[:, kt : kt + 2, ms],
                        hwT[:, kt : kt + 2, :],
                        start=(kt == 0),
                        stop=(kt + 2 >= D // 128),
                        perf_mode=DR,
                    )
                sc_e = wp.tile([128, 2048], BF, tag="sc_e")
                nc.scalar.activation(
                    sc_e[:, :WH],
                    ps[:],
                    ACTF.Exp,
                    scale=HID_DESCALE,
                    accum_out=res[:, m, 0:1],
                )
                sc_t = wp.tile([128, WH], BF, tag="sc_t")
                nc.vector.scalar_tensor_tensor(
                    out=sc_t[:],
                    in0=iota[:],
                    scalar=rels[:, m, 0:1],
                    in1=ps[:],
                    op0=OP.is_equal,
                    op1=OP.mult,
                    accum_out=res[:, m, 3:4],
                )

            def tail_group(lhsT, w2, kdim, m, gw, goff, s_ap):
                # fp8 DoubleRow GEMM group + exp/accum partial sum
                ms = slice(m * 128, (m + 1) * 128)
                ps = pslot(gw)
                nsub = kdim // 128
                for co in range(0, gw, 512):
                    cw = min(512, gw - co)
                    for kt in range(0, nsub, 2):
                        nc.tensor.matmul(
                            ps[:, co : co + cw],
                            lhsT[:, kt : kt + 2, ms],
                            w2[:, kt : kt + 2, goff + co : goff + co + cw],
                            start=(kt == 0),
                            stop=(kt + 2 >= nsub),
                            perf_mode=DR,
                        )
                sc_e = wp.tile([128, 2048], BF, tag="sc_e")
                nc.scalar.activation(
                    sc_e[:, :gw], ps[:], ACTF.Exp, scale=DESCALE, accum_out=s_ap
                )

            def transposes(hT, hn, hdim):
                # batched XBAR transpose hT[h, r] -> hn[r, h]:
                # out[p, j, q] = in[q, j*128+p]
                for kt in range(hdim // 128):
                    nc.sync.dma_start_transpose(
                        hn[:, :, kt * 128 : (kt + 1) * 128], hT[:, kt, :]
                    )

            def dot(hn, wg, hdim, m, t_ap):
                sc_d = wp.tile([128, H0], BF, tag="sc_d")
                nc.vector.scalar_tensor_tensor(
                    out=sc_d[:, :hdim],
                    in0=hn[:, m, :],
                    scalar=1.0,
                    in1=wg[:, m, :],
                    op0=OP.mult,
                    op1=OP.mult,
                    accum_out=t_ap,
                )

            # emission order feeds ACT as early as possible:
            # head -> h0 hidden -> tail0 -> h1 hidden -> tail1
            with nc.named_scope("head_hidden0"):
                for i in range(H0 // 128):
                    for m in range(4 * i, 4 * i + 4):
                        head_group(m)
                    hidden_block(h0T, h0T8, w1t0, H0, i, fslot)
            fpool_cm.__exit__(None, None, None)
            psp_cm = tc.tile_pool(name="psum", bufs=2, space="PSUM")
            psp = psp_cm.__enter__()
            transposes(h0T, h0n, H0)
            with nc.named_scope("tail0_hidden1"):
                for m in range(MT):
                    tail_group(h0T8, w2t0, H0, m, W0, 0, res[:, m, 1:2])
                    dot(h0n, wg0, H0, m, res[:, m, 4:5])
                    if m in (6, 13):
                        hidden_block(h1T, h1T8, w1t1, H1, m == 13, pslot)
            transposes(h1T, h1n, H1)
            with nc.named_scope("tail1"):
                for m in range(MT):
                    spart = pp.tile([128, 2], F32, tag="spart")
                    dot(h1n, wg1, H1, m, res[:, m, 5:6])
                    # B group first: exp on ACT without accum, sum on DVE,
                    # so the final ACT exp (A group) overlaps the B reduce
                    ms = slice(m * 128, (m + 1) * 128)
                    ps = pslot(1792)
                    for co in range(0, 1792, 512):
                        cw = min(512, 1792 - co)
                        nc.tensor.matmul(
                            ps[:, co : co + cw],
                            h1T8[:, 0:2, ms],
                            w2t1[:, 0:2, 2048 + co : 2048 + co + cw],
                            start=True,
                            stop=True,
                            perf_mode=DR,
                        )
                    sc_e = wp.tile([128, 2048], BF, tag="sc_e")
                    nc.scalar.activation(
                        sc_e[:, :1792], ps[:], ACTF.Exp, scale=DESCALE
                    )
                    nc.vector.reduce_sum(spart[:, 1:2], sc_e[:, :1792], axis=AX.X)
                    tail_group(h1T8, w2t1, H1, m, 2048, 0, spart[:, 0:1])
                    nc.vector.reduce_sum(res[:, m, 2:3], spart[:], axis=AX.X)

            psp_cm.__exit__(None, None, None)
            nc.sync.dma_start(res_d[:], res[:])

    nc.finalize()
    return nc


def _get_nc():
    global _CACHED_NC
    if _CACHED_NC is None:
        _CACHED_NC = _build_nc()
    return _CACHED_NC


def _tiled(a2d):
    """[K, F] (K multiple of 128) -> contiguous [128, K//128, F]."""
    K, F = a2d.shape
    return np.ascontiguousarray(
        a2d.reshape(K // 128, 128, F).transpose(1, 0, 2)
    )


def _pm(vec):
    """[N] -> [128, MT] with [p, m] = vec[m*128+p]."""
    return np.ascontiguousarray(vec.reshape(MT, 128).T)


def _unpm(a):
    """[128, MT] -> [N]."""
    return np.ascontiguousarray(a.T).reshape(N)


def make_in_maps(inp, tgt, head_w, t0_w1, t0_w2, t1_w1, t1_w2):
    inp = np.asarray(inp, dtype=np.float32)
    tgt = np.asarray(tgt).astype(np.int64)

    inpT = _tiled((inp.T * IN_SCALE).astype(FP8))
    w1t0 = _tiled((np.asarray(t0_w1, np.float32).T * W1_SCALE).astype(FP8))
    w1t1 = _tiled((np.asarray(t1_w1, np.float32).T * W1_SCALE).astype(FP8))

    hwT_full = np.zeros((D, HEAD_PAD), FP8)
    hwT_full[:, :HEAD] = (np.asarray(head_w, np.float32).T * W1_SCALE).astype(FP8)
    w2t0_full = (np.asarray(t0_w2, np.float32).T * W_SCALE).astype(FP8)
    w2t1_full = np.zeros((H1, T1_PAD), FP8)
    w2t1_full[:, :T1] = (np.asarray(t1_w2, np.float32).T * W_SCALE).astype(FP8)

    iota = np.broadcast_to(
        np.arange(WH, dtype=np.float32)[None, :], (128, WH)
    ).copy()

    gi = np.where(tgt < C0, tgt, np.where(tgt < C1, C0, C0 + 1))
    rel0 = tgt - C0
    rel1 = tgt - C1

    # host-gathered target weight rows (bf16, matching device operand
    # precision), zeroed on cores that don't own the target's column shard
    t0_w2_bf = np.asarray(t0_w2, np.float32).astype(BF16)
    t1_w2_bf = np.asarray(t1_w2, np.float32).astype(BF16)

    def _gather_rows(tbl, row, own):
        g = tbl[np.clip(row, 0, tbl.shape[0] - 1)]
        g[~own] = 0
        return np.ascontiguousarray(
            g.reshape(MT, 128, tbl.shape[1]).transpose(1, 0, 2)
        )

    in_maps = []
    for i in range(NCORES):
        in_maps.append(
            {
                "inpT": inpT,
                "w1t0": w1t0,
                "w1t1": w1t1,
                "hwT": _tiled(hwT_full[:, i * WH : (i + 1) * WH]),
                "w2t0": _tiled(w2t0_full[:, i * W0 : (i + 1) * W0]),
                "w2t1": _tiled(w2t1_full[:, i * W1 : (i + 1) * W1]),
                "wg0": _gather_rows(t0_w2_bf, rel0, (rel0 // W0) == i),
                "wg1": _gather_rows(t1_w2_bf, rel1, (rel1 // W1) == i),
                "iota": iota,
                "rels": np.stack(
                    [
                        _pm((gi - i * WH).astype(np.float32)),
                        _pm((rel0 - i * W0).astype(np.float32)),
                        _pm((rel1 - i * W1).astype(np.float32)),
                    ],
                    axis=2,
                ).copy(),
            }
        )
    return in_maps, tgt


def combine(results, tgt):
    """results: list of per-core {'res': [128, MT, 6]} -> final [N] f32 NLL."""
    S = np.zeros((3, N), np.float64)
    T = np.zeros((3, N), np.float64)
    for r in results:
        res = np.asarray(r["res"], np.float64)
        for c in range(3):
            S[c] += _unpm(res[:, :, c])
            T[c] += _unpm(res[:, :, 3 + c])
    S[0] -= PAD_H  # zero-padded columns contribute exp(0)=1 each (core 7)
    S[1] -= PAD_0
    S[2] -= PAD_1

    in1 = (tgt >= C0) & (tgt < C1)
    in2 = tgt >= C1
    head_term = T[0] * HID_DESCALE - np.log(S[0])
    lp0 = T[1] - np.log(S[1])
    lp1 = T[2] - np.log(S[2])
    out = head_term + np.where(in1, lp0, 0.0) + np.where(in2, lp1, 0.0)
    return (-out).astype(np.float32)


def kernel(inp, tgt, head_w, t0_w1, t0_w2, t1_w1, t1_w2):
    global LAST_RESULT
    nc = _get_nc()
    in_maps, tgt64 = make_in_maps(inp, tgt, head_w, t0_w1, t0_w2, t1_w1, t1_w2)
    out = run_bass_kernel_spmd(
        nc, in_maps, core_ids=list(range(NCORES)), trace=TRACE
    )
    LAST_RESULT = out
    return combine(out.results, tgt64)



# revision 4
# speedup vs baseline: 1.3555x; 1.3555x over previous
"""Trainium2 Bass kernel for AdaptiveLogSoftmaxWithLoss (moe_routing).

Sharding: class columns are tensor-sharded 8 ways (head 4002->4096 so each
core gets 512, tail0 16000 -> 2000/core, tail1 30257->30720 -> 3840/core);
every core runs an identical SPMD program over the sample batches with 1/8
of the output classes.

The adaptive part: only samples whose target falls in a tail cluster need
that cluster's GEMM + log-softmax (masked rows contribute 0 in the
reference).  The host packs the ~655 cluster-0 rows into 6 sample tiles and
the ~1238 cluster-1 rows into 10 tiles; the tail GEMMs, exps and target
dots run only on those packed batches, cutting PE streaming ~45% and the
scalar-engine exp stream ~42% vs computing all 2048 rows.

Per core:
  - warmup at t=0: a zero-filled fp8 tile feeds dummy DoubleRow matmuls to
    lift the PE HAM clock gate to 2.4 GHz, and a dummy exp forces the ACT
    table load, both during the input DMA,
  - hidden projections h0T=[512,768], h1T=[256,1280] for the packed rows
    (fp8 DoubleRow GEMMs), descale to bf16 + requant to fp8,
  - head logits in [sample, class] supergroups of 4 m-tiles sharing one
    4-bank PSUM tile: one 2048-wide exp (scale-folded descale) per
    supergroup, per-row sums via a DVE reduce over the [128,4,512] view,
    target logits via the (iota==rel)*logit DVE pass per m-tile,
  - tail logit shards as fp8 DoubleRow GEMM groups (<=2048-wide PSUM),
    one exp+accum_out per group -> per-row partial sum-exp; logits are
    small (|x| < ~4) so no max subtraction is needed,
  - tail target logits: bf16 dots of XBAR-transposed hidden rows against
    host-gathered target weight rows (zeroed on non-owner cores),
  - emission interleaves ACT-heavy tail groups with PE-heavy head/hidden
    groups so both engines stay fed; input DMAs are split across the two
    HWDGE rings (sync + scalar).

Host combine: sum partials over cores, subtract exp(0)=1 for zero-padded
columns, lse = log(sum), scatter packed tail terms back by sample index,
NLL = -(head + masked tail terms) as in the reference.
"""

import numpy as np
import ml_dtypes

import concourse.bass as bass
import concourse.bacc as bacc
import concourse.mybir as mybir
import concourse.tile as tile
from concourse.bass_utils import run_bass_kernel_spmd

BF16 = ml_dtypes.bfloat16
FP8 = ml_dtypes.float8_e4m3
H_SCALE = 8.0     # h cast to fp8 at 8x
W_SCALE = 64.0    # tail w2 cast to fp8 at 64x
IN_SCALE = 16.0   # inp cast to fp8 at 16x
W1_SCALE = 64.0   # w1 / head_w cast to fp8 at 64x
HID_DESCALE = 1.0 / (IN_SCALE * W1_SCALE)
DESCALE = 1.0 / (H_SCALE * W_SCALE)
NCORES = 8
N, D = 2048, 1024
H0, H1 = 512, 256
C0, C1 = 4000, 20000
HEAD = 4002        # 4000 shortlist + 2 cluster-logit columns
HEAD_PAD = 4096    # padded so 8 cores get 512 each
T0 = 16000
T1 = 30257
T1_PAD = 30720     # padded so 8 cores get 3840 each
WH, W0, W1 = HEAD_PAD // 8, T0 // 8, T1_PAD // 8     # 512, 2000, 3840
MT = N // 128                                        # 16 sample tiles
PAD_H = HEAD_PAD - HEAD   # 94 zero columns, all on core 7
PAD_1 = T1_PAD - T1       # 463 zero columns, all on core 7
NT0 = 6                   # packed cluster-0 sample tiles (655 rows used)
NT1 = 10                  # packed cluster-1 sample tiles (1238 rows used)

# module-level knobs for test.py (harness never touches these)
TRACE = False
LAST_RESULT = None

_CACHED_NC = None
_CACHED_CAP = None


def _build_nc(nt0, nt1):
    np0, np1 = nt0 * 128, nt1 * 128
    nrow = 16 + nt0 + nt1
    nc = bacc.Bacc(None)
    BF = mybir.dt.bfloat16
    F8 = mybir.dt.float8e4
    F32 = mybir.dt.float32
    AX = mybir.AxisListType
    OP = mybir.AluOpType
    ACTF = mybir.ActivationFunctionType
    DR = mybir.MatmulPerfMode.DoubleRow

    inpT_d = nc.dram_tensor("inpT", [128, D // 128, N], F8, kind="ExternalInput")
    inpT0_d = nc.dram_tensor("inpT0", [128, D // 128, np0], F8, kind="ExternalInput")
    inpT1_d = nc.dram_tensor("inpT1", [128, D // 128, np1], F8, kind="ExternalInput")
    hwT_d = nc.dram_tensor("hwT", [128, D // 128, WH], F8, kind="ExternalInput")
    w1t0_d = nc.dram_tensor("w1t0", [128, D // 128, H0], F8, kind="ExternalInput")
    w1t1_d = nc.dram_tensor("w1t1", [128, D // 128, H1], F8, kind="ExternalInput")
    w2t0_d = nc.dram_tensor("w2t0", [128, H0 // 128, W0], F8, kind="ExternalInput")
    w2t1_d = nc.dram_tensor("w2t1", [128, H1 // 128, W1], F8, kind="ExternalInput")
    wg0_d = nc.dram_tensor("wg0", [128, nt0, H0], BF, kind="ExternalInput")
    wg1_d = nc.dram_tensor("wg1", [128, nt1, H1], BF, kind="ExternalInput")
    iota_d = nc.dram_tensor("iota", [128, WH], F32, kind="ExternalInput")
    rels_d = nc.dram_tensor("rels", [128, MT, 1], F32, kind="ExternalInput")
    zer_d = nc.dram_tensor("zer", [128, 2, 640], F8, kind="ExternalInput")
    zf_d = nc.dram_tensor("zf", [128, 16], F32, kind="ExternalInput")
    res_d = nc.dram_tensor("res", [128, nrow, 3], F32, kind="ExternalOutput")

    with tile.TileContext(nc) as tc:
        with (
            tc.tile_pool(name="const", bufs=1) as cp,
            tc.tile_pool(name="work", bufs=3) as wp,
            tc.tile_pool(name="psum", bufs=2, space="PSUM") as psp,
        ):
            inpT = cp.tile([128, D // 128, N], F8)
            inpT0 = cp.tile([128, D // 128, np0], F8)
            inpT1 = cp.tile([128, D // 128, np1], F8)
            hwT = cp.tile([128, D // 128, WH], F8)
            w1t0 = cp.tile([128, D // 128, H0], F8)
            w1t1 = cp.tile([128, D // 128, H1], F8)
            w2t0 = cp.tile([128, H0 // 128, W0], F8)
            w2t1 = cp.tile([128, H1 // 128, W1], F8)
            wg0 = cp.tile([128, nt0, H0], BF)
            wg1 = cp.tile([128, nt1, H1], BF)
            iota = cp.tile([128, WH], F32)
            rels = cp.tile([128, MT, 1], F32)
            zer = cp.tile([128, 2, 640], F8)
            zf = cp.tile([128, 16], F32)
            h0T = cp.tile([128, H0 // 128, np0], BF)
            h1T = cp.tile([128, H1 // 128, np1], BF)
            h0T8 = cp.tile([128, H0 // 128, np0], F8)
            h1T8 = cp.tile([128, H1 // 128, np1], F8)
            h0n = cp.tile([128, nt0, H0], BF)
            h1n = cp.tile([128, nt1, H1], BF)
            res = cp.tile([128, nrow, 3], F32)

            # warmup inputs first (tiny), then the two DMA rings:
            # sync ring feeds the hidden/tail path, scalar ring the head path.
            nc.scalar.dma_start(zer[:], zer_d[:])
            nc.scalar.dma_start(zf[:], zf_d[:])
            nc.sync.dma_start(w1t0[:], w1t0_d[:])
            nc.sync.dma_start(inpT0[:], inpT0_d[:])
            nc.scalar.dma_start(hwT[:], hwT_d[:])
            for kt in range(D // 128):
                nc.scalar.dma_start(inpT[:, kt], inpT_d[:, kt])
            nc.scalar.dma_start(iota[:], iota_d[:])
            nc.scalar.dma_start(rels[:], rels_d[:])
            nc.sync.dma_start(w1t1[:], w1t1_d[:])
            nc.sync.dma_start(inpT1[:], inpT1_d[:])
            nc.sync.dma_start(w2t0[:], w2t0_d[:])
            nc.sync.dma_start(wg0[:], wg0_d[:])
            nc.sync.dma_start(w2t1[:], w2t1_d[:])
            nc.sync.dma_start(wg1[:], wg1_d[:])

            def pslot(w):
                ps = psp.tile([128, 2048], F32, tag="ps", name="ps")
                return ps[:, :w]

            def pslot3():
                return psp.tile([128, 4, WH], F32, tag="ps", name="ps3")

            with nc.named_scope("warmup"):
                nc.vector.memset(res[:], 0.0)
                # dummy exp pulls the ACT table load off the critical path
                sc_z = wp.tile([128, 16], BF, tag="sc_z")
                nc.scalar.activation(sc_z[:], zf[:], ACTF.Exp)
                # ~10 x 512-col zero matmuls lift the HAM clock gate while
                # the real inputs stream in
                ps = pslot(512)
                for _ in range(10):
                    nc.tensor.matmul(
                        ps[:], zer[:, :, :128], zer[:, :, 128:640],
                        start=True, stop=True, perf_mode=DR,
                    )

            def hidden_unit(hT, hT8, w1, inpTp, mh, chunks):
                ps = pslot(2048)
                for co, cw in chunks:
                    for kt in range(0, D // 128, 2):
                        nc.tensor.matmul(
                            ps[:, co : co + cw],
                            w1[:, kt : kt + 2, mh * 128 : (mh + 1) * 128],
                            inpTp[:, kt : kt + 2, co : co + cw],
                            start=(kt == 0),
                            stop=(kt + 2 >= D // 128),
                            perf_mode=DR,
                        )
                for co, cw in chunks:
                    nc.vector.tensor_scalar_mul(
                        hT[:, mh, co : co + cw], ps[:, co : co + cw], HID_DESCALE
                    )
                    nc.vector.tensor_scalar_mul(
                        hT8[:, mh, co : co + cw], hT[:, mh, co : co + cw], H_SCALE
                    )

            def head_sg(sg):
                ps = pslot3()
                for g in range(4):
                    m = 4 * sg + g
                    ms = slice(m * 128, (m + 1) * 128)
                    for kt in range(0, D // 128, 2):
                        nc.tensor.matmul(
                            ps[:, g],
                            inpT[:, kt : kt + 2, ms],
                            hwT[:, kt : kt + 2, :],
                            start=(kt == 0),
                            stop=(kt + 2 >= D // 128),
                            perf_mode=DR,
                        )
                sc_e = wp.tile([128, 4, WH], BF, tag="sc_e")
                nc.scalar.activation(sc_e[:], ps[:], ACTF.Exp, scale=HID_DESCALE)
                nc.vector.reduce_sum(
                    res[:, 4 * sg : 4 * sg + 4, 0:1], sc_e[:], axis=AX.X
                )
                for g in range(4):
                    m = 4 * sg + g
                    sc_t = wp.tile([128, WH], BF, tag="sc_t")
                    nc.vector.scalar_tensor_tensor(
                        out=sc_t[:],
                        in0=iota[:],
                        scalar=rels[:, m, 0:1],
                        in1=ps[:, g],
                        op0=OP.is_equal,
                        op1=OP.mult,
                        accum_out=res[:, m, 1:2],
                    )

            T0CH = ((0, 512), (512, 512), (1024, 512), (1536, W0 - 1536))

            def t0_unit(j):
                ms = slice(j * 128, (j + 1) * 128)
                ps = pslot(W0)
                for co, cw in T0CH:
                    for kt in range(0, H0 // 128, 2):
                        nc.tensor.matmul(
                            ps[:, co : co + cw],
                            h0T8[:, kt : kt + 2, ms],
                            w2t0[:, kt : kt + 2, co : co + cw],
                            start=(kt == 0),
                            stop=(kt + 2 >= H0 // 128),
                            perf_mode=DR,
                        )
                sc_e = wp.tile([128, 4 * WH], BF, tag="sc_e", name="sc_e")
                nc.scalar.activation(
                    sc_e[:, :W0], ps[:], ACTF.Exp,
                    scale=DESCALE, accum_out=res[:, 16 + j, 0:1],
                )
                sc_d = wp.tile([128, H0], BF, tag="sc_d")
                nc.vector.scalar_tensor_tensor(
                    out=sc_d[:],
                    in0=h0n[:, j, :],
                    scalar=1.0,
                    in1=wg0[:, j, :],
                    op0=OP.mult,
                    op1=OP.mult,
                    accum_out=res[:, 16 + j, 1:2],
                )

            T1CHA = ((0, 512), (512, 512), (1024, 512), (1536, 512))
            T1CHB = ((0, 512), (512, 512), (1024, 512), (1536, 256))

            def t1_unit(j):
                ms = slice(j * 128, (j + 1) * 128)
                for half, chunks, goff in ((0, T1CHA, 0), (1, T1CHB, 2048)):
                    gw = sum(c[1] for c in chunks)
                    ps = pslot(gw)
                    for co, cw in chunks:
                        nc.tensor.matmul(
                            ps[:, co : co + cw],
                            h1T8[:, 0:2, ms],
                            w2t1[:, 0:2, goff + co : goff + co + cw],
                            start=True,
                            stop=True,
                            perf_mode=DR,
                        )
                    sc_e = wp.tile([128, 4 * WH], BF, tag="sc_e", name="sc_e")
                    nc.scalar.activation(
                        sc_e[:, :gw], ps[:], ACTF.Exp,
                        scale=DESCALE,
                        accum_out=res[:, 16 + nt0 + j, half : half + 1],
                    )
                sc_d = wp.tile([128, H0], BF, tag="sc_d")
                nc.vector.scalar_tensor_tensor(
                    out=sc_d[:, :H1],
                    in0=h1n[:, j, :],
                    scalar=1.0,
                    in1=wg1[:, j, :],
                    op0=OP.mult,
                    op1=OP.mult,
                    accum_out=res[:, 16 + nt0 + j, 2:3],
                )

            H0CH = ((0, 512), (512, np0 - 512)) if np0 > 512 else ((0, np0),)
            h1c = [(c * 512, min(512, np1 - c * 512)) for c in range((np1 + 511) // 512)]

            # emission order interleaves ACT-heavy tail units with PE-heavy
            # head/hidden units so the exp stream never starves
            with nc.named_scope("front"):
                for mh in range(H0 // 128):
                    hidden_unit(h0T, h0T8, w1t0, inpT0, mh, H0CH)
                    nc.scalar.dma_start_transpose(
                        h0n[:, :, mh * 128 : (mh + 1) * 128], h0T[:, mh, :]
                    )
                head_sg(0)
                t0_unit(0)
                hidden_unit(h1T, h1T8, w1t1, inpT1, 0, h1c)
                nc.scalar.dma_start_transpose(h1n[:, :, 0:128], h1T[:, 0, :])
                t0_unit(1)
                hidden_unit(h1T, h1T8, w1t1, inpT1, 1, h1c)
                nc.scalar.dma_start_transpose(h1n[:, :, 128:256], h1T[:, 1, :])
            with nc.named_scope("mid"):
                t1_unit(0)
                t0_unit(2)
                t1_unit(1)
                t1_unit(2)
                head_sg(1)
                t1_unit(3)
                t0_unit(3)
                t1_unit(4)
                t1_unit(5)
                head_sg(2)
            with nc.named_scope("tail"):
                t1_unit(6)
                t0_unit(4)
                t1_unit(7)
                t0_unit(5)
                t1_unit(8)
                t1_unit(9)
                head_sg(3)

            nc.sync.dma_start(res_d[:], res[:])

    nc.finalize()
    return nc


def _get_nc(nt0, nt1):
    global _CACHED_NC, _CACHED_CAP
    if _CACHED_NC is None or _CACHED_CAP[0] < nt0 or _CACHED_CAP[1] < nt1:
        cap = (max(nt0, NT0), max(nt1, NT1))
        _CACHED_NC = _build_nc(*cap)
        _CACHED_CAP = cap
    return _CACHED_NC, _CACHED_CAP


def _tiled(a2d):
    """[K, F] (K multiple of 128) -> contiguous [128, K//128, F]."""
    K, F = a2d.shape
    return np.ascontiguousarray(
        a2d.reshape(K // 128, 128, F).transpose(1, 0, 2)
    )


def _pm(vec):
    """[M*128] -> [128, M] with [p, m] = vec[m*128+p]."""
    M = vec.shape[0] // 128
    return np.ascontiguousarray(vec.reshape(M, 128).T)


def _unpm(a):
    """[128, M] -> [M*128]."""
    return np.ascontiguousarray(a.T).reshape(-1)


def _pack(idx, ntiles):
    """Pad an index list to ntiles*128 entries (repeating a valid index)."""
    cap = ntiles * 128
    out = np.zeros(cap, dtype=np.int64)
    out[: len(idx)] = idx
    if len(idx) < cap:
        out[len(idx):] = idx[0] if len(idx) else 0
    return out


def make_in_maps(inp, tgt, head_w, t0_w1, t0_w2, t1_w1, t1_w2, nt0, nt1):
    inp = np.asarray(inp, dtype=np.float32)
    tgt = np.asarray(tgt).astype(np.int64)

    in0 = tgt < C0
    in1 = (tgt >= C0) & (tgt < C1)
    in2 = tgt >= C1
    pidx0 = _pack(np.where(in1)[0], nt0)
    pidx1 = _pack(np.where(in2)[0], nt1)

    inpT = _tiled((inp.T * IN_SCALE).astype(FP8))
    inpT0 = _tiled((inp[pidx0].T * IN_SCALE).astype(FP8))
    inpT1 = _tiled((inp[pidx1].T * IN_SCALE).astype(FP8))
    w1t0 = _tiled((np.asarray(t0_w1, np.float32).T * W1_SCALE).astype(FP8))
    w1t1 = _tiled((np.asarray(t1_w1, np.float32).T * W1_SCALE).astype(FP8))

    hwT_full = np.zeros((D, HEAD_PAD), FP8)
    hwT_full[:, :HEAD] = (np.asarray(head_w, np.float32).T * W1_SCALE).astype(FP8)
    w2t0_full = (np.asarray(t0_w2, np.float32).T * W_SCALE).astype(FP8)
    w2t1_full = np.zeros((H1, T1_PAD), FP8)
    w2t1_full[:, :T1] = (np.asarray(t1_w2, np.float32).T * W_SCALE).astype(FP8)

    iota = np.broadcast_to(
        np.arange(WH, dtype=np.float32)[None, :], (128, WH)
    ).copy()

    gi = np.where(in0, tgt, np.where(in1, C0, C0 + 1))
    rel0 = tgt[pidx0] - C0
    rel1 = tgt[pidx1] - C1

    # host-gathered target weight rows (bf16, matching device operand
    # precision), zeroed on cores that don't own the target's column shard
    t0_w2_bf = np.asarray(t0_w2, np.float32).astype(BF16)
    t1_w2_bf = np.asarray(t1_w2, np.float32).astype(BF16)

    def _gather_rows(tbl, row, own, ntiles):
        g = tbl[np.clip(row, 0, tbl.shape[0] - 1)]
        g[~own] = 0
        return np.ascontiguousarray(
            g.reshape(ntiles, 128, tbl.shape[1]).transpose(1, 0, 2)
        )

    zer = np.zeros((128, 2, 640), FP8)
    zf = np.zeros((128, 16), np.float32)

    in_maps = []
    for i in range(NCORES):
        in_maps.append(
            {
                "inpT": inpT,
                "inpT0": inpT0,
                "inpT1": inpT1,
                "w1t0": w1t0,
                "w1t1": w1t1,
                "hwT": _tiled(hwT_full[:, i * WH : (i + 1) * WH]),
                "w2t0": _tiled(w2t0_full[:, i * W0 : (i + 1) * W0]),
                "w2t1": _tiled(w2t1_full[:, i * W1 : (i + 1) * W1]),
                "wg0": _gather_rows(t0_w2_bf, rel0, (rel0 // W0) == i, nt0),
                "wg1": _gather_rows(t1_w2_bf, rel1, (rel1 // W1) == i, nt1),
                "iota": iota,
                "rels": _pm((gi - i * WH).astype(np.float32))[:, :, None].copy(),
                "zer": zer,
                "zf": zf,
            }
        )
    return in_maps, tgt, pidx0, pidx1


def combine(results, tgt, pidx0, pidx1, nt0, nt1):
    """results: list of per-core {'res': [128, nrow, 3]} -> final [N] f32."""
    acc = np.zeros_like(np.asarray(results[0]["res"], np.float64))
    for r in results:
        acc += np.asarray(r["res"], np.float64)

    in1 = (tgt >= C0) & (tgt < C1)
    in2 = tgt >= C1
    n1, n2 = int(in1.sum()), int(in2.sum())

    S_head = _unpm(acc[:, 0:16, 0]) - PAD_H
    T_head = _unpm(acc[:, 0:16, 1]) * HID_DESCALE
    head_term = T_head - np.log(S_head)

    S0 = _unpm(acc[:, 16 : 16 + nt0, 0])
    T0v = _unpm(acc[:, 16 : 16 + nt0, 1])
    lp0 = T0v - np.log(S0)

    S1 = _unpm(acc[:, 16 + nt0 :, 0] + acc[:, 16 + nt0 :, 1]) - PAD_1
    T1v = _unpm(acc[:, 16 + nt0 :, 2])
    lp1 = T1v - np.log(S1)

    out = head_term
    add0 = np.zeros(N)
    add0[pidx0[:n1]] = lp0[:n1]
    add1 = np.zeros(N)
    add1[pidx1[:n2]] = lp1[:n2]
    out = out + add0 + add1
    return (-out).astype(np.float32)


def kernel(inp, tgt, head_w, t0_w1, t0_w2, t1_w1, t1_w2):
    global LAST_RESULT
    tgt64 = np.asarray(tgt).astype(np.int64)
    n1 = int(((tgt64 >= C0) & (tgt64 < C1)).sum())
    n2 = int((tgt64 >= C1).sum())
    nt0 = max(1, -(-n1 // 128))
    nt1 = max(1, -(-n2 // 128))
    nc, (nt0, nt1) = _get_nc(nt0, nt1)
    in_maps, tgt64, pidx0, pidx1 = make_in_maps(
        inp, tgt, head_w, t0_w1, t0_w2, t1_w1, t1_w2, nt0, nt1
    )
    out = run_bass_kernel_spmd(
        nc, in_maps, core_ids=list(range(NCORES)), trace=TRACE
    )
    LAST_RESULT = out
    return combine(out.results, tgt64, pidx0, pidx1, nt0, nt1)


# revision 11
# speedup vs baseline: 1.5060x; 1.1110x over previous
"""Trainium2 Bass kernel for AdaptiveLogSoftmaxWithLoss (moe_routing).

Sharding: class columns are tensor-sharded 8 ways (head 4002->4096 so each
core gets 512, tail0 16000 -> 2000/core, tail1 30257->30720 -> 3840/core);
every core runs an identical SPMD program over the sample batches with 1/8
of the output classes.

The adaptive part: only samples whose target falls in a tail cluster need
that cluster's GEMM + log-softmax (masked rows contribute 0 in the
reference).  The host packs the ~655 cluster-0 rows into 6 sample tiles and
the ~1238 cluster-1 rows into 10 tiles; the tail GEMMs, exps and target
dots run only on those packed batches, cutting PE streaming ~45% and the
scalar-engine exp stream ~42% vs computing all 2048 rows.

Per core:
  - warmup at t=0: a zero-filled fp8 tile feeds dummy DoubleRow matmuls to
    lift the PE HAM clock gate to 2.4 GHz, and a dummy exp forces the ACT
    table load, both during the input DMA,
  - hidden projections h0T=[512,768], h1T=[256,1280] for the packed rows
    (fp8 DoubleRow GEMMs), descale to bf16 + requant to fp8,
  - head logits in [sample, class] supergroups of 4 m-tiles sharing one
    4-bank PSUM tile: one 2048-wide exp (scale-folded descale) per
    supergroup, per-row sums via a DVE reduce over the [128,4,512] view,
    target logits via the (iota==rel)*logit DVE pass per m-tile,
  - tail logit shards as fp8 DoubleRow GEMM groups (<=2048-wide PSUM),
    one exp+accum_out per group -> per-row partial sum-exp; logits are
    small (|x| < ~4) so no max subtraction is needed,
  - tail target logits: bf16 dots of XBAR-transposed hidden rows against
    host-gathered target weight rows (zeroed on non-owner cores),
  - emission interleaves ACT-heavy tail groups with PE-heavy head/hidden
    groups so both engines stay fed; input DMAs are split across the two
    HWDGE rings (sync + scalar).

Host combine: sum partials over cores, subtract exp(0)=1 for zero-padded
columns, lse = log(sum), scatter packed tail terms back by sample index,
NLL = -(head + masked tail terms) as in the reference.
"""

import numpy as np
import ml_dtypes

import concourse.bass as bass
import concourse.bacc as bacc
import concourse.mybir as mybir
import concourse.tile as tile
from concourse.bass_utils import run_bass_kernel_spmd

BF16 = ml_dtypes.bfloat16
FP8 = ml_dtypes.float8_e4m3
H_SCALE = 8.0     # h cast to fp8 at 8x
W_SCALE = 64.0    # tail w2 cast to fp8 at 64x
IN_SCALE = 16.0   # inp cast to fp8 at 16x
W1_SCALE = 64.0   # w1 / head_w cast to fp8 at 64x
HID_DESCALE = 1.0 / (IN_SCALE * W1_SCALE)
DESCALE = 1.0 / (H_SCALE * W_SCALE)
NCORES = 8
N, D = 2048, 1024
H0, H1 = 512, 256
C0, C1 = 4000, 20000
HEAD = 4002        # 4000 shortlist + 2 cluster-logit columns
HEAD_PAD = 4096    # padded so 8 cores get 512 each
T0 = 16000
T1 = 30257
T1_PAD = 30720     # padded so 8 cores get 3840 each
WH, W0, W1 = HEAD_PAD // 8, T0 // 8, T1_PAD // 8     # 512, 2000, 3840
MT = N // 128                                        # 16 sample tiles
PAD_H = HEAD_PAD - HEAD   # 94 zero columns, all on core 7
PAD_1 = T1_PAD - T1       # 463 zero columns, all on core 7
NT0 = 6                   # packed cluster-0 sample tiles (655 rows used)
NT1 = 10                  # packed cluster-1 sample tiles (1238 rows used)

# module-level knobs for test.py (harness never touches these)
TRACE = False
LAST_RESULT = None

_CACHED_NC = None
_CACHED_CAP = None


def _build_nc(nt0, nt1):
    np0, np1 = nt0 * 128, nt1 * 128
    nrow = 16 + nt0 + nt1
    nc = bacc.Bacc(None)
    BF = mybir.dt.bfloat16
    F8 = mybir.dt.float8e4
    F32 = mybir.dt.float32
    AX = mybir.AxisListType
    OP = mybir.AluOpType
    ACTF = mybir.ActivationFunctionType
    DR = mybir.MatmulPerfMode.DoubleRow

    inpT_d = nc.dram_tensor("inpT", [128, D // 128, N], F8, kind="ExternalInput")
    inpT0_d = nc.dram_tensor("inpT0", [128, D // 128, np0], F8, kind="ExternalInput")
    inpT1_d = nc.dram_tensor("inpT1", [128, D // 128, np1], F8, kind="ExternalInput")
    hwT_d = nc.dram_tensor("hwT", [128, D // 128, WH], F8, kind="ExternalInput")
    w1t0_d = nc.dram_tensor("w1t0", [128, D // 128, H0], F8, kind="ExternalInput")
    w1t1_d = nc.dram_tensor("w1t1", [128, D // 128, H1], F8, kind="ExternalInput")
    w2t0_d = nc.dram_tensor("w2t0", [128, H0 // 128, W0], F8, kind="ExternalInput")
    w2t1_d = nc.dram_tensor("w2t1", [128, H1 // 128, W1], F8, kind="ExternalInput")
    wg0_d = nc.dram_tensor("wg0", [128, nt0, H0], BF, kind="ExternalInput")
    wg1_d = nc.dram_tensor("wg1", [128, nt1, H1], BF, kind="ExternalInput")
    rels_d = nc.dram_tensor("rels", [128, MT, 1], F32, kind="ExternalInput")
    res_d = nc.dram_tensor("res", [128, nrow, 3], F32, kind="ExternalOutput")

    with tile.TileContext(nc) as tc:
        with (
            tc.tile_pool(name="const", bufs=1) as cp,
            tc.tile_pool(name="work", bufs=3) as wp,
            tc.tile_pool(name="psum", bufs=2, space="PSUM") as psp,
        ):
            inpT = cp.tile([128, D // 128, N], F8)
            inpT0 = cp.tile([128, D // 128, np0], F8)
            inpT1 = cp.tile([128, D // 128, np1], F8)
            hwT = cp.tile([128, D // 128, WH], F8)
            w1t0 = cp.tile([128, D // 128, H0], F8)
            w1t1 = cp.tile([128, D // 128, H1], F8)
            w2t0 = cp.tile([128, H0 // 128, W0], F8)
            w2t1 = cp.tile([128, H1 // 128, W1], F8)
            wg0 = cp.tile([128, nt0, H0], BF)
            wg1 = cp.tile([128, nt1, H1], BF)
            iota = cp.tile([128, WH], F32)
            rels = cp.tile([128, MT, 1], F32)
            zer = cp.tile([128, 2, 640], F8)
            zf = cp.tile([128, 16], F32)
            nc.vector.memset(zer[:], 0.0)
            nc.vector.memset(zf[:], 0.0)
            nc.gpsimd.iota(
                iota[:], [[1, WH]], base=0, channel_multiplier=0,
                allow_small_or_imprecise_dtypes=True,
            )
            h0T = cp.tile([128, H0 // 128, np0], BF)
            h1T = cp.tile([128, H1 // 128, np1], BF)
            h0T8 = cp.tile([128, H0 // 128, np0], F8)
            h1T8 = cp.tile([128, H1 // 128, np1], F8)
            h0n = cp.tile([128, nt0, H0], BF)
            h1n = cp.tile([128, nt1, H1], BF)
            res = cp.tile([128, nrow, 3], F32)

            # single HWDGE ring (sync), ordered by when compute needs each
            # tensor; the scalar queue stays free for the exp stream
            nc.sync.dma_start(w1t0[:], w1t0_d[:])
            nc.sync.dma_start(inpT0[:], inpT0_d[:])
            nc.sync.dma_start(w2t0[:], w2t0_d[:])
            nc.sync.dma_start(hwT[:], hwT_d[:])
            for kt in range(D // 128):
                nc.sync.dma_start(inpT[:, kt], inpT_d[:, kt])
            nc.sync.dma_start(rels[:], rels_d[:])
            nc.sync.dma_start(w1t1[:], w1t1_d[:])
            nc.sync.dma_start(inpT1[:], inpT1_d[:])
            nc.sync.dma_start(w2t1[:], w2t1_d[:])
            nc.sync.dma_start(wg0[:], wg0_d[:])
            nc.sync.dma_start(wg1[:], wg1_d[:])

            def pslot(w):
                ps = psp.tile([128, 2048], F32, tag="ps", name="ps")
                return ps[:, :w]

            def pslot3():
                return psp.tile([128, 4, WH], F32, tag="ps", name="ps3")

            with nc.named_scope("warmup"):
                nc.vector.memset(res[:], 0.0)
                # dummy exp pulls the ACT table load off the critical path
                sc_z = wp.tile([128, 16], BF, tag="sc_z")
                nc.scalar.activation(sc_z[:], zf[:], ACTF.Exp)
                # ~10 x 512-col zero matmuls lift the HAM clock gate while
                # the real inputs stream in
                ps = pslot(512)
                for _ in range(10):
                    nc.tensor.matmul(
                        ps[:], zer[:, :, :128], zer[:, :, 128:640],
                        start=True, stop=True, perf_mode=DR,
                    )

            def hidden_unit(hT, hT8, w1, inpTp, mh, chunks):
                ps = pslot(2048)
                for co, cw in chunks:
                    for kt in range(0, D // 128, 2):
                        nc.tensor.matmul(
                            ps[:, co : co + cw],
                            w1[:, kt : kt + 2, mh * 128 : (mh + 1) * 128],
                            inpTp[:, kt : kt + 2, co : co + cw],
                            start=(kt == 0),
                            stop=(kt + 2 >= D // 128),
                            perf_mode=DR,
                        )
                for co, cw in chunks:
                    nc.vector.tensor_scalar_mul(
                        hT[:, mh, co : co + cw], ps[:, co : co + cw], HID_DESCALE
                    )
                    nc.vector.tensor_scalar_mul(
                        hT8[:, mh, co : co + cw], hT[:, mh, co : co + cw], H_SCALE
                    )

            def head_sg(sg):
                ps = pslot3()
                for g in range(4):
                    m = 4 * sg + g
                    ms = slice(m * 128, (m + 1) * 128)
                    for kt in range(0, D // 128, 2):
                        nc.tensor.matmul(
                            ps[:, g],
                            inpT[:, kt : kt + 2, ms],
                            hwT[:, kt : kt + 2, :],
                            start=(kt == 0),
                            stop=(kt + 2 >= D // 128),
                            perf_mode=DR,
                        )
                sc_e = wp.tile([128, 4, WH], BF, tag="sc_e")
                nc.scalar.activation(sc_e[:], ps[:], ACTF.Exp, scale=HID_DESCALE)
                nc.vector.reduce_sum(
                    res[:, 4 * sg : 4 * sg + 4, 0:1], sc_e[:], axis=AX.X
                )
                for g in range(4):
                    m = 4 * sg + g
                    sc_t = wp.tile([128, WH], BF, tag="sc_t")
                    nc.vector.scalar_tensor_tensor(
                        out=sc_t[:],
                        in0=iota[:],
                        scalar=rels[:, m, 0:1],
                        in1=ps[:, g],
                        op0=OP.is_equal,
                        op1=OP.mult,
                        accum_out=res[:, m, 1:2],
                    )

            T0CH = ((0, 512), (512, 512), (1024, 512), (1536, W0 - 1536))

            def t0_unit(j):
                ms = slice(j * 128, (j + 1) * 128)
                ps = pslot(W0)
                for co, cw in T0CH:
                    for kt in range(0, H0 // 128, 2):
                        nc.tensor.matmul(
                            ps[:, co : co + cw],
                            h0T8[:, kt : kt + 2, ms],
                            w2t0[:, kt : kt + 2, co : co + cw],
                            start=(kt == 0),
                            stop=(kt + 2 >= H0 // 128),
                            perf_mode=DR,
                        )
                sc_e = wp.tile([128, 4 * WH], BF, tag="sc_e", name="sc_e")
                nc.scalar.activation(
                    sc_e[:, :W0], ps[:], ACTF.Exp,
                    scale=DESCALE, accum_out=res[:, 16 + j, 0:1],
                )
                sc_d = wp.tile([128, H0], BF, tag="sc_d")
                nc.vector.scalar_tensor_tensor(
                    out=sc_d[:],
                    in0=h0n[:, j, :],
                    scalar=1.0,
                    in1=wg0[:, j, :],
                    op0=OP.mult,
                    op1=OP.mult,
                    accum_out=res[:, 16 + j, 1:2],
                )

            T1CHA = ((0, 512), (512, 512), (1024, 512), (1536, 512))
            T1CHB = ((0, 512), (512, 512), (1024, 512), (1536, 256))

            def t1_unit(j):
                ms = slice(j * 128, (j + 1) * 128)
                for half, chunks, goff in ((0, T1CHA, 0), (1, T1CHB, 2048)):
                    gw = sum(c[1] for c in chunks)
                    ps = pslot(gw)
                    for co, cw in chunks:
                        nc.tensor.matmul(
                            ps[:, co : co + cw],
                            h1T8[:, 0:2, ms],
                            w2t1[:, 0:2, goff + co : goff + co + cw],
                            start=True,
                            stop=True,
                            perf_mode=DR,
                        )
                    sc_e = wp.tile([128, 4 * WH], BF, tag="sc_e", name="sc_e")
                    nc.scalar.activation(
                        sc_e[:, :gw], ps[:], ACTF.Exp,
                        scale=DESCALE,
                        accum_out=res[:, 16 + nt0 + j, half : half + 1],
                    )
                sc_d = wp.tile([128, H0], BF, tag="sc_d")
                nc.vector.scalar_tensor_tensor(
                    out=sc_d[:, :H1],
                    in0=h1n[:, j, :],
                    scalar=1.0,
                    in1=wg1[:, j, :],
                    op0=OP.mult,
                    op1=OP.mult,
                    accum_out=res[:, 16 + nt0 + j, 2:3],
                )

            H0CH = ((0, 512), (512, np0 - 512)) if np0 > 512 else ((0, np0),)
            h1c = [(c * 512, min(512, np1 - c * 512)) for c in range((np1 + 511) // 512)]

            # emission order interleaves ACT-heavy tail units with PE-heavy
            # head/hidden units so the exp stream never starves
            with nc.named_scope("front"):
                for mh in range(H0 // 128):
                    hidden_unit(h0T, h0T8, w1t0, inpT0, mh, H0CH)
                    nc.sync.dma_start_transpose(
                        h0n[:, :, mh * 128 : (mh + 1) * 128], h0T[:, mh, :]
                    )
                t0_unit(0)
                t0_unit(1)
                head_sg(0)
                hidden_unit(h1T, h1T8, w1t1, inpT1, 0, h1c)
                nc.sync.dma_start_transpose(h1n[:, :, 0:128], h1T[:, 0, :])
                t0_unit(2)
                hidden_unit(h1T, h1T8, w1t1, inpT1, 1, h1c)
                nc.sync.dma_start_transpose(h1n[:, :, 128:256], h1T[:, 1, :])
                t0_unit(3)
            with nc.named_scope("mid"):
                t1_unit(0)
                head_sg(1)
                t1_unit(1)
                t0_unit(4)
                t1_unit(2)
                head_sg(2)
                t1_unit(3)
                t0_unit(5)
                t1_unit(4)
            with nc.named_scope("tail"):
                t1_unit(5)
                t1_unit(6)
                head_sg(3)
                t1_unit(7)
                t1_unit(8)
                t1_unit(9)

            nc.sync.dma_start(res_d[:], res[:])

    nc.finalize()
    return nc


def _get_nc(nt0, nt1):
    global _CACHED_NC, _CACHED_CAP
    if _CACHED_NC is None or _CACHED_CAP[0] < nt0 or _CACHED_CAP[1] < nt1:
        cap = (max(nt0, NT0), max(nt1, NT1))
        _CACHED_NC = _build_nc(*cap)
        _CACHED_CAP = cap
    return _CACHED_NC, _CACHED_CAP


def _tiled(a2d):
    """[K, F] (K multiple of 128) -> contiguous [128, K//128, F]."""
    K, F = a2d.shape
    return np.ascontiguousarray(
        a2d.reshape(K // 128, 128, F).transpose(1, 0, 2)
    )


def _pm(vec):
    """[M*128] -> [128, M] with [p, m] = vec[m*128+p]."""
    M = vec.shape[0] // 128
    return np.ascontiguousarray(vec.reshape(M, 128).T)


def _unpm(a):
    """[128, M] -> [M*128]."""
    return np.ascontiguousarray(a.T).reshape(-1)


def _pack(idx, ntiles):
    """Pad an index list to ntiles*128 entries (repeating a valid index)."""
    cap = ntiles * 128
    out = np.zeros(cap, dtype=np.int64)
    out[: len(idx)] = idx
    if len(idx) < cap:
        out[len(idx):] = idx[0] if len(idx) else 0
    return out


def make_in_maps(inp, tgt, head_w, t0_w1, t0_w2, t1_w1, t1_w2, nt0, nt1):
    inp = np.asarray(inp, dtype=np.float32)
    tgt = np.asarray(tgt).astype(np.int64)

    in0 = tgt < C0
    in1 = (tgt >= C0) & (tgt < C1)
    in2 = tgt >= C1
    pidx0 = _pack(np.where(in1)[0], nt0)
    pidx1 = _pack(np.where(in2)[0], nt1)

    inpT = _tiled((inp.T * IN_SCALE).astype(FP8))
    inpT0 = _tiled((inp[pidx0].T * IN_SCALE).astype(FP8))
    inpT1 = _tiled((inp[pidx1].T * IN_SCALE).astype(FP8))
    w1t0 = _tiled((np.asarray(t0_w1, np.float32).T * W1_SCALE).astype(FP8))
    w1t1 = _tiled((np.asarray(t1_w1, np.float32).T * W1_SCALE).astype(FP8))

    hwT_full = np.zeros((D, HEAD_PAD), FP8)
    hwT_full[:, :HEAD] = (np.asarray(head_w, np.float32).T * W1_SCALE).astype(FP8)
    w2t0_full = (np.asarray(t0_w2, np.float32).T * W_SCALE).astype(FP8)
    w2t1_full = np.zeros((H1, T1_PAD), FP8)
    w2t1_full[:, :T1] = (np.asarray(t1_w2, np.float32).T * W_SCALE).astype(FP8)

    gi = np.where(in0, tgt, np.where(in1, C0, C0 + 1))
    rel0 = tgt[pidx0] - C0
    rel1 = tgt[pidx1] - C1

    # host-gathered target weight rows (bf16, matching device operand
    # precision), zeroed on cores that don't own the target's column shard
    t0_w2_bf = np.asarray(t0_w2, np.float32).astype(BF16)
    t1_w2_bf = np.asarray(t1_w2, np.float32).astype(BF16)

    def _gather_rows(tbl, row, own, ntiles):
        g = tbl[np.clip(row, 0, tbl.shape[0] - 1)]
        g[~own] = 0
        return np.ascontiguousarray(
            g.reshape(ntiles, 128, tbl.shape[1]).transpose(1, 0, 2)
        )

    in_maps = []
    for i in range(NCORES):
        in_maps.append(
            {
                "inpT": inpT,
                "inpT0": inpT0,
                "inpT1": inpT1,
                "w1t0": w1t0,
                "w1t1": w1t1,
                "hwT": _tiled(hwT_full[:, i * WH : (i + 1) * WH]),
                "w2t0": _tiled(w2t0_full[:, i * W0 : (i + 1) * W0]),
                "w2t1": _tiled(w2t1_full[:, i * W1 : (i + 1) * W1]),
                "wg0": _gather_rows(t0_w2_bf, rel0, (rel0 // W0) == i, nt0),
                "wg1": _gather_rows(t1_w2_bf, rel1, (rel1 // W1) == i, nt1),
                "rels": _pm((gi - i * WH).astype(np.float32))[:, :, None].copy(),
            }
        )
    return in_maps, tgt, pidx0, pidx1


def combine(results, tgt, pidx0, pidx1, nt0, nt1):
    """results: list of per-core {'res': [128, nrow, 3]} -> final [N] f32."""
    acc = np.zeros_like(np.asarray(results[0]["res"], np.float64))
    for r in results:
        acc += np.asarray(r["res"], np.float64)

    in1 = (tgt >= C0) & (tgt < C1)
    in2 = tgt >= C1
    n1, n2 = int(in1.sum()), int(in2.sum())

    S_head = _unpm(acc[:, 0:16, 0]) - PAD_H
    T_head = _unpm(acc[:, 0:16, 1]) * HID_DESCALE
    head_term = T_head - np.log(S_head)

    S0 = _unpm(acc[:, 16 : 16 + nt0, 0])
    T0v = _unpm(acc[:, 16 : 16 + nt0, 1])
    lp0 = T0v - np.log(S0)

    S1 = _unpm(acc[:, 16 + nt0 :, 0] + acc[:, 16 + nt0 :, 1]) - PAD_1
    T1v = _unpm(acc[:, 16 + nt0 :, 2])
    lp1 = T1v - np.log(S1)

    out = head_term
    add0 = np.zeros(N)
    add0[pidx0[:n1]] = lp0[:n1]
    add1 = np.zeros(N)
    add1[pidx1[:n2]] = lp1[:n2]
    out = out + add0 + add1
    return (-out).astype(np.float32)


def kernel(inp, tgt, head_w, t0_w1, t0_w2, t1_w1, t1_w2):
    global LAST_RESULT
    tgt64 = np.asarray(tgt).astype(np.int64)
    n1 = int(((tgt64 >= C0) & (tgt64 < C1)).sum())
    n2 = int((tgt64 >= C1).sum())
    nt0 = max(1, -(-n1 // 128))
    nt1 = max(1, -(-n2 // 128))
    nc, (nt0, nt1) = _get_nc(nt0, nt1)
    in_maps, tgt64, pidx0, pidx1 = make_in_maps(
        inp, tgt, head_w, t0_w1, t0_w2, t1_w1, t1_w2, nt0, nt1
    )
    out = run_bass_kernel_spmd(
        nc, in_maps, core_ids=list(range(NCORES)), trace=TRACE
    )
    LAST_RESULT = out
    return combine(out.results, tgt64, pidx0, pidx1, nt0, nt1)


# revision 15
# speedup vs baseline: 1.6046x; 1.0655x over previous
"""Trainium2 Bass kernel for AdaptiveLogSoftmaxWithLoss (moe_routing).

Sharding: class columns are tensor-sharded 8 ways (head 4002->4096 so each
core gets 512, tail0 16000 -> 2000/core, tail1 30257->30720 -> 3840/core);
every core runs an identical SPMD program over the sample batches with 1/8
of the output classes.

The adaptive part: only samples whose target falls in a tail cluster need
that cluster's GEMM + log-softmax (masked rows contribute 0 in the
reference).  The host packs the ~655 cluster-0 rows into 6 sample tiles and
the ~1238 cluster-1 rows into 10 tiles; the tail GEMMs, exps and target
dots run only on those packed batches, cutting PE streaming ~45% and the
scalar-engine exp stream ~42% vs computing all 2048 rows.

Per core:
  - warmup at t=0: a zero-filled fp8 tile feeds dummy DoubleRow matmuls to
    lift the PE HAM clock gate to 2.4 GHz, and a dummy exp forces the ACT
    table load, both during the input DMA,
  - hidden projections h0T=[512,768], h1T=[256,1280] for the packed rows
    (fp8 DoubleRow GEMMs), descale to bf16 + requant to fp8,
  - head logits in [sample, class] supergroups of 4 m-tiles sharing one
    4-bank PSUM tile: one 2048-wide exp (scale-folded descale) per
    supergroup, per-row sums via a DVE reduce over the [128,4,512] view,
    target logits via the (iota==rel)*logit DVE pass per m-tile,
  - tail logit shards as fp8 DoubleRow GEMM groups (<=2048-wide PSUM),
    one exp+accum_out per group -> per-row partial sum-exp; logits are
    small (|x| < ~4) so no max subtraction is needed,
  - tail target logits: bf16 dots of XBAR-transposed hidden rows against
    host-gathered target weight rows (zeroed on non-owner cores),
  - emission interleaves ACT-heavy tail groups with PE-heavy head/hidden
    groups so both engines stay fed; input DMAs are split across the two
    HWDGE rings (sync + scalar).

Host combine: sum partials over cores, subtract exp(0)=1 for zero-padded
columns, lse = log(sum), scatter packed tail terms back by sample index,
NLL = -(head + masked tail terms) as in the reference.
"""

import numpy as np
import ml_dtypes

import concourse.bass as bass
import concourse.bacc as bacc
import concourse.mybir as mybir
import concourse.tile as tile
from concourse.bass_utils import run_bass_kernel_spmd

BF16 = ml_dtypes.bfloat16
FP8 = ml_dtypes.float8_e4m3
H_SCALE = 8.0     # h cast to fp8 at 8x
W_SCALE = 64.0    # tail w2 cast to fp8 at 64x
IN_SCALE = 16.0   # inp cast to fp8 at 16x
W1_SCALE = 64.0   # w1 / head_w cast to fp8 at 64x
HID_DESCALE = 1.0 / (IN_SCALE * W1_SCALE)
DESCALE = 1.0 / (H_SCALE * W_SCALE)
NCORES = 8
N, D = 2048, 1024
H0, H1 = 512, 256
C0, C1 = 4000, 20000
HEAD = 4002        # 4000 shortlist + 2 cluster-logit columns
HEAD_PAD = 4096    # padded so 8 cores get 512 each
T0 = 16000
T1 = 30257
T1_PAD = 30720     # padded so 8 cores get 3840 each
WH, W0, W1 = HEAD_PAD // 8, T0 // 8, T1_PAD // 8     # 512, 2000, 3840
MT = N // 128                                        # 16 sample tiles
PAD_H = HEAD_PAD - HEAD   # 94 zero columns, all on core 7
PAD_1 = T1_PAD - T1       # 463 zero columns, all on core 7
NT0 = 6                   # packed cluster-0 sample tiles (655 rows used)
NT1 = 10                  # packed cluster-1 sample tiles (1238 rows used)

# module-level knobs for test.py (harness never touches these)
TRACE = False
LAST_RESULT = None

_CACHED_NC = None
_CACHED_CAP = None


def _build_nc(nt0, nt1):
    np0, np1 = nt0 * 128, nt1 * 128
    nrow = 16 + nt0 + nt1
    nc = bacc.Bacc(None)
    BF = mybir.dt.bfloat16
    F8 = mybir.dt.float8e4
    F32 = mybir.dt.float32
    AX = mybir.AxisListType
    OP = mybir.AluOpType
    ACTF = mybir.ActivationFunctionType
    DR = mybir.MatmulPerfMode.DoubleRow

    inpT_d = nc.dram_tensor("inpT", [128, D // 128, N], F8, kind="ExternalInput")
    inpT0_d = nc.dram_tensor("inpT0", [128, D // 128, np0], F8, kind="ExternalInput")
    inpT1_d = nc.dram_tensor("inpT1", [128, D // 128, np1], F8, kind="ExternalInput")
    hwT_d = nc.dram_tensor("hwT", [128, D // 128, WH], F8, kind="ExternalInput")
    w1t0_d = nc.dram_tensor("w1t0", [128, D // 128, H0], F8, kind="ExternalInput")
    w1t1_d = nc.dram_tensor("w1t1", [128, D // 128, H1], F8, kind="ExternalInput")
    w2t0_d = nc.dram_tensor("w2t0", [128, H0 // 128, W0], F8, kind="ExternalInput")
    w2t1_d = nc.dram_tensor("w2t1", [128, H1 // 128, W1], F8, kind="ExternalInput")
    wg0_d = nc.dram_tensor("wg0", [128, nt0, H0], BF, kind="ExternalInput")
    wg1_d = nc.dram_tensor("wg1", [128, nt1, H1], BF, kind="ExternalInput")
    rels_d = nc.dram_tensor("rels", [128, MT, 1], F32, kind="ExternalInput")
    res_d = nc.dram_tensor("res", [128, nrow, 3], F32, kind="ExternalOutput")

    with tile.TileContext(nc) as tc:
        with (
            tc.tile_pool(name="const", bufs=1) as cp,
            tc.tile_pool(name="work", bufs=3) as wp,
            tc.tile_pool(name="psum", bufs=2, space="PSUM") as psp,
        ):
            inpT = cp.tile([128, D // 128, N], F8)
            inpT0 = cp.tile([128, D // 128, np0], F8)
            inpT1 = cp.tile([128, D // 128, np1], F8)
            hwT = cp.tile([128, D // 128, WH], F8)
            w1t0 = cp.tile([128, D // 128, H0], F8)
            w1t1 = cp.tile([128, D // 128, H1], F8)
            w2t0 = cp.tile([128, H0 // 128, W0], F8)
            w2t1 = cp.tile([128, H1 // 128, W1], F8)
            wg0 = cp.tile([128, nt0, H0], BF)
            wg1 = cp.tile([128, nt1, H1], BF)
            iota = cp.tile([128, WH], F32)
            rels = cp.tile([128, MT, 1], F32)
            zer = cp.tile([128, 2, 640], F8)
            zf = cp.tile([128, 16], F32)
            nc.vector.memset(zer[:], 0.0)
            nc.vector.memset(zf[:], 0.0)
            nc.gpsimd.iota(
                iota[:], [[1, WH]], base=0, channel_multiplier=0,
                allow_small_or_imprecise_dtypes=True,
            )
            h0T = cp.tile([128, H0 // 128, np0], BF)
            h1T = cp.tile([128, H1 // 128, np1], BF)
            h0T8 = cp.tile([128, H0 // 128, np0], F8)
            h1T8 = cp.tile([128, H1 // 128, np1], F8)
            h0n = cp.tile([128, nt0, H0], BF)
            h1n = cp.tile([128, nt1, H1], BF)
            res = cp.tile([128, nrow, 3], F32)

            # single HWDGE ring (sync), ordered by when compute needs each
            # tensor; the scalar queue stays free for the exp stream.
            # w1t0/inpT0 come in halves so the first hidden matmuls (which
            # only need k-tiles 0-3, via subtile deps) start ~2us earlier.
            nc.sync.dma_start(w1t0[:, 0:4], w1t0_d[:, 0:4])
            nc.sync.dma_start(inpT0[:, 0:4], inpT0_d[:, 0:4])
            nc.sync.dma_start(w1t0[:, 4:8], w1t0_d[:, 4:8])
            nc.sync.dma_start(inpT0[:, 4:8], inpT0_d[:, 4:8])
            nc.sync.dma_start(w2t0[:], w2t0_d[:])
            nc.sync.dma_start(hwT[:], hwT_d[:])
            for kt in range(D // 128):
                nc.sync.dma_start(inpT[:, kt], inpT_d[:, kt])
            nc.sync.dma_start(rels[:], rels_d[:])
            nc.sync.dma_start(w1t1[:], w1t1_d[:])
            nc.sync.dma_start(inpT1[:], inpT1_d[:])
            nc.sync.dma_start(w2t1[:], w2t1_d[:])
            nc.sync.dma_start(wg0[:], wg0_d[:])
            nc.sync.dma_start(wg1[:], wg1_d[:])

            def pslot(w):
                ps = psp.tile([128, 2048], F32, tag="ps", name="ps")
                return ps[:, :w]

            def pslot3():
                return psp.tile([128, 4, WH], F32, tag="ps", name="ps3")

            with nc.named_scope("warmup"):
                nc.vector.memset(res[:], 0.0)
                # dummy exp pulls the ACT table load off the critical path
                sc_z = wp.tile([128, 16], BF, tag="sc_z")
                nc.scalar.activation(sc_z[:], zf[:], ACTF.Exp)
                # ~10 x 512-col zero matmuls lift the HAM clock gate while
                # the real inputs stream in
                ps = pslot(512)
                for _ in range(10):
                    nc.tensor.matmul(
                        ps[:], zer[:, :, :128], zer[:, :, 128:640],
                        start=True, stop=True, perf_mode=DR,
                    )

            def hidden_unit(hT, hT8, w1, inpTp, mh, chunks):
                ps = pslot(2048)
                for co, cw in chunks:
                    for kt in range(0, D // 128, 2):
                        nc.tensor.matmul(
                            ps[:, co : co + cw],
                            w1[:, kt : kt + 2, mh * 128 : (mh + 1) * 128],
                            inpTp[:, kt : kt + 2, co : co + cw],
                            start=(kt == 0),
                            stop=(kt + 2 >= D // 128),
                            perf_mode=DR,
                        )
                for co, cw in chunks:
                    nc.vector.tensor_scalar_mul(
                        hT[:, mh, co : co + cw], ps[:, co : co + cw], HID_DESCALE
                    )
                    nc.vector.tensor_scalar_mul(
                        hT8[:, mh, co : co + cw], hT[:, mh, co : co + cw], H_SCALE
                    )

            def head_sg(sg):
                ps = pslot3()
                for g in range(4):
                    m = 4 * sg + g
                    ms = slice(m * 128, (m + 1) * 128)
                    for kt in range(0, D // 128, 2):
                        nc.tensor.matmul(
                            ps[:, g],
                            inpT[:, kt : kt + 2, ms],
                            hwT[:, kt : kt + 2, :],
                            start=(kt == 0),
                            stop=(kt + 2 >= D // 128),
                            perf_mode=DR,
                        )
                sc_e = wp.tile([128, 4, WH], BF, tag="sc_e")
                nc.scalar.activation(sc_e[:], ps[:], ACTF.Exp, scale=HID_DESCALE)
                nc.vector.reduce_sum(
                    res[:, 4 * sg : 4 * sg + 4, 0:1], sc_e[:], axis=AX.X
                )
                # gather exp(target logit) from the SBUF exp output rather
                # than the PSUM logits: frees the PSUM slot right after the
                # exp, so the next GEMM group isn't blocked on these STTs
                # (host takes log of the gathered value)
                for g in range(4):
                    m = 4 * sg + g
                    sc_t = wp.tile([128, WH], BF, tag="sc_t")
                    nc.vector.scalar_tensor_tensor(
                        out=sc_t[:],
                        in0=iota[:],
                        scalar=rels[:, m, 0:1],
                        in1=sc_e[:, g],
                        op0=OP.is_equal,
                        op1=OP.mult,
                        accum_out=res[:, m, 1:2],
                    )

            T0CH = ((0, 512), (512, 512), (1024, 512), (1536, W0 - 1536))

            def t0_unit(j):
                ms = slice(j * 128, (j + 1) * 128)
                ps = pslot(W0)
                for co, cw in T0CH:
                    for kt in range(0, H0 // 128, 2):
                        nc.tensor.matmul(
                            ps[:, co : co + cw],
                            h0T8[:, kt : kt + 2, ms],
                            w2t0[:, kt : kt + 2, co : co + cw],
                            start=(kt == 0),
                            stop=(kt + 2 >= H0 // 128),
                            perf_mode=DR,
                        )
                sc_e = wp.tile([128, 4 * WH], BF, tag="sc_e", name="sc_e")
                nc.scalar.activation(
                    sc_e[:, :W0], ps[:], ACTF.Exp,
                    scale=DESCALE, accum_out=res[:, 16 + j, 0:1],
                )
                sc_d = wp.tile([128, H0], BF, tag="sc_d")
                nc.vector.scalar_tensor_tensor(
                    out=sc_d[:],
                    in0=h0n[:, j, :],
                    scalar=1.0,
                    in1=wg0[:, j, :],
                    op0=OP.mult,
                    op1=OP.mult,
                    accum_out=res[:, 16 + j, 1:2],
                )

            T1CHA = ((0, 512), (512, 512), (1024, 512), (1536, 512))
            T1CHB = ((0, 512), (512, 512), (1024, 512), (1536, 256))

            def t1_unit(j):
                ms = slice(j * 128, (j + 1) * 128)
                for half, chunks, goff in ((0, T1CHA, 0), (1, T1CHB, 2048)):
                    gw = sum(c[1] for c in chunks)
                    ps = pslot(gw)
                    for co, cw in chunks:
                        nc.tensor.matmul(
                            ps[:, co : co + cw],
                            h1T8[:, 0:2, ms],
                            w2t1[:, 0:2, goff + co : goff + co + cw],
                            start=True,
                            stop=True,
                            perf_mode=DR,
                        )
                    sc_e = wp.tile([128, 4 * WH], BF, tag="sc_e", name="sc_e")
                    nc.scalar.activation(
                        sc_e[:, :gw], ps[:], ACTF.Exp,
                        scale=DESCALE,
                        accum_out=res[:, 16 + nt0 + j, half : half + 1],
                    )
                sc_d = wp.tile([128, H0], BF, tag="sc_d")
                nc.vector.scalar_tensor_tensor(
                    out=sc_d[:, :H1],
                    in0=h1n[:, j, :],
                    scalar=1.0,
                    in1=wg1[:, j, :],
                    op0=OP.mult,
                    op1=OP.mult,
                    accum_out=res[:, 16 + nt0 + j, 2:3],
                )

            H0CH = ((0, 512), (512, np0 - 512)) if np0 > 512 else ((0, np0),)
            h1c = [(c * 512, min(512, np1 - c * 512)) for c in range((np1 + 511) // 512)]

            # emission order interleaves ACT-heavy tail units with PE-heavy
            # head/hidden units so the exp stream never starves
            with nc.named_scope("front"):
                for mh in range(H0 // 128):
                    hidden_unit(h0T, h0T8, w1t0, inpT0, mh, H0CH)
                    nc.sync.dma_start_transpose(
                        h0n[:, :, mh * 128 : (mh + 1) * 128], h0T[:, mh, :]
                    )
                t0_unit(0)
                t0_unit(1)
                head_sg(0)
                hidden_unit(h1T, h1T8, w1t1, inpT1, 0, h1c)
                nc.sync.dma_start_transpose(h1n[:, :, 0:128], h1T[:, 0, :])
                t0_unit(2)
                hidden_unit(h1T, h1T8, w1t1, inpT1, 1, h1c)
                nc.sync.dma_start_transpose(h1n[:, :, 128:256], h1T[:, 1, :])
                t0_unit(3)
            with nc.named_scope("mid"):
                t1_unit(0)
                head_sg(1)
                t1_unit(1)
                t0_unit(4)
                t1_unit(2)
                head_sg(2)
                t1_unit(3)
                t0_unit(5)
                t1_unit(4)
            with nc.named_scope("tail"):
                t1_unit(5)
                t1_unit(6)
                t1_unit(7)
                t1_unit(8)
                t1_unit(9)
                head_sg(3)

            nc.sync.dma_start(res_d[:], res[:])

    nc.finalize()
    return nc


def _get_nc(nt0, nt1):
    global _CACHED_NC, _CACHED_CAP
    if _CACHED_NC is None or _CACHED_CAP[0] < nt0 or _CACHED_CAP[1] < nt1:
        cap = (max(nt0, NT0), max(nt1, NT1))
        _CACHED_NC = _build_nc(*cap)
        _CACHED_CAP = cap
    return _CACHED_NC, _CACHED_CAP


def _tiled(a2d):
    """[K, F] (K multiple of 128) -> contiguous [128, K//128, F]."""
    K, F = a2d.shape
    return np.ascontiguousarray(
        a2d.reshape(K // 128, 128, F).transpose(1, 0, 2)
    )


def _pm(vec):
    """[M*128] -> [128, M] with [p, m] = vec[m*128+p]."""
    M = vec.shape[0] // 128
    return np.ascontiguousarray(vec.reshape(M, 128).T)


def _unpm(a):
    """[128, M] -> [M*128]."""
    return np.ascontiguousarray(a.T).reshape(-1)


def _pack(idx, ntiles):
    """Pad an index list to ntiles*128 entries (repeating a valid index)."""
    cap = ntiles * 128
    out = np.zeros(cap, dtype=np.int64)
    out[: len(idx)] = idx
    if len(idx) < cap:
        out[len(idx):] = idx[0] if len(idx) else 0
    return out


def make_in_maps(inp, tgt, head_w, t0_w1, t0_w2, t1_w1, t1_w2, nt0, nt1):
    inp = np.asarray(inp, dtype=np.float32)
    tgt = np.asarray(tgt).astype(np.int64)

    in0 = tgt < C0
    in1 = (tgt >= C0) & (tgt < C1)
    in2 = tgt >= C1
    pidx0 = _pack(np.where(in1)[0], nt0)
    pidx1 = _pack(np.where(in2)[0], nt1)

    inpT = _tiled((inp.T * IN_SCALE).astype(FP8))
    inpT0 = _tiled((inp[pidx0].T * IN_SCALE).astype(FP8))
    inpT1 = _tiled((inp[pidx1].T * IN_SCALE).astype(FP8))
    w1t0 = _tiled((np.asarray(t0_w1, np.float32).T * W1_SCALE).astype(FP8))
    w1t1 = _tiled((np.asarray(t1_w1, np.float32).T * W1_SCALE).astype(FP8))

    hwT_full = np.zeros((D, HEAD_PAD), FP8)
    hwT_full[:, :HEAD] = (np.asarray(head_w, np.float32).T * W1_SCALE).astype(FP8)
    w2t0_full = (np.asarray(t0_w2, np.float32).T * W_SCALE).astype(FP8)
    w2t1_full = np.zeros((H1, T1_PAD), FP8)
    w2t1_full[:, :T1] = (np.asarray(t1_w2, np.float32).T * W_SCALE).astype(FP8)

    gi = np.where(in0, tgt, np.where(in1, C0, C0 + 1))
    rel0 = tgt[pidx0] - C0
    rel1 = tgt[pidx1] - C1

    # host-gathered target weight rows (bf16, matching device operand
    # precision), zeroed on cores that don't own the target's column shard
    t0_w2_bf = np.asarray(t0_w2, np.float32).astype(BF16)
    t1_w2_bf = np.asarray(t1_w2, np.float32).astype(BF16)

    def _gather_rows(tbl, row, own, ntiles):
        g = tbl[np.clip(row, 0, tbl.shape[0] - 1)]
        g[~own] = 0
        return np.ascontiguousarray(
            g.reshape(ntiles, 128, tbl.shape[1]).transpose(1, 0, 2)
        )

    in_maps = []
    for i in range(NCORES):
        in_maps.append(
            {
                "inpT": inpT,
                "inpT0": inpT0,
                "inpT1": inpT1,
                "w1t0": w1t0,
                "w1t1": w1t1,
                "hwT": _tiled(hwT_full[:, i * WH : (i + 1) * WH]),
                "w2t0": _tiled(w2t0_full[:, i * W0 : (i + 1) * W0]),
                "w2t1": _tiled(w2t1_full[:, i * W1 : (i + 1) * W1]),
                "wg0": _gather_rows(t0_w2_bf, rel0, (rel0 // W0) == i, nt0),
                "wg1": _gather_rows(t1_w2_bf, rel1, (rel1 // W1) == i, nt1),
                "rels": _pm((gi - i * WH).astype(np.float32))[:, :, None].copy(),
            }
        )
    return in_maps, tgt, pidx0, pidx1


def combine(results, tgt, pidx0, pidx1, nt0, nt1):
    """results: list of per-core {'res': [128, nrow, 3]} -> final [N] f32."""
    acc = np.zeros_like(np.asarray(results[0]["res"], np.float64))
    for r in results:
        acc += np.asarray(r["res"], np.float64)

    in1 = (tgt >= C0) & (tgt < C1)
    in2 = tgt >= C1
    n1, n2 = int(in1.sum()), int(in2.sum())

    S_head = _unpm(acc[:, 0:16, 0]) - PAD_H
    T_head = np.log(_unpm(acc[:, 0:16, 1]))  # gathered exp(target logit)
    head_term = T_head - np.log(S_head)

    S0 = _unpm(acc[:, 16 : 16 + nt0, 0])
    T0v = _unpm(acc[:, 16 : 16 + nt0, 1])
    lp0 = T0v - np.log(S0)

    S1 = _unpm(acc[:, 16 + nt0 :, 0] + acc[:, 16 + nt0 :, 1]) - PAD_1
    T1v = _unpm(acc[:, 16 + nt0 :, 2])
    lp1 = T1v - np.log(S1)

    out = head_term
    add0 = np.zeros(N)
    add0[pidx0[:n1]] = lp0[:n1]
    add1 = np.zeros(N)
    add1[pidx1[:n2]] = lp1[:n2]
    out = out + add0 + add1
    return (-out).astype(np.float32)


def kernel(inp, tgt, head_w, t0_w1, t0_w2, t1_w1, t1_w2):
    global LAST_RESULT
    tgt64 = np.asarray(tgt).astype(np.int64)
    n1 = int(((tgt64 >= C0) & (tgt64 < C1)).sum())
    n2 = int((tgt64 >= C1).sum())
    nt0 = max(1, -(-n1 // 128))
    nt1 = max(1, -(-n2 // 128))
    nc, (nt0, nt1) = _get_nc(nt0, nt1)
    in_maps, tgt64, pidx0, pidx1 = make_in_maps(
        inp, tgt, head_w, t0_w1, t0_w2, t1_w1, t1_w2, nt0, nt1
    )
    out = run_bass_kernel_spmd(
        nc, in_maps, core_ids=list(range(NCORES)), trace=TRACE
    )
    LAST_RESULT = out
    return combine(out.results, tgt64, pidx0, pidx1, nt0, nt1)
